# revision 29
# baseline (speedup 1.0000x reference)
"""Trainium2 Bass kernel for nn_Attention_37598143709539.

Dense transformer attention with a 1x1-conv relative positional bias:
  qkv = x @ Wqkv ; per-head scores = q k^T * scale + conv1x1(centroid_delta)
  out = softmax(scores) @ v ; final = concat-heads @ Wout + bout

Distribution: pure data-parallel over (batch, query-half) -> 8 cores; core
cid handles batch cid//2, query rows [cid%2*1024, +1024).  Keys/values and
the softmax run over the full 2048-key axis locally, so no collectives are
needed; the host concatenates the 8 output shards.

On-core layout: feature-major (transposed) activations throughout:
  scoresT[j, i] = k_h^T q_h   (key token j on partitions, query i free)
  p = exp(scoresT + biasT - C) (C=5 shift keeps the unnormalized sums in
                                f16 range; it cancels in the normalizer)
  poT[dh|1, i] accumulated with lhsT = [v_h | ones]: the ones column gives
  the softmax normalizer row for free; outT chains into Wout.

Per [128,1024] scores plane the 3-channel conv bias is applied by one of
two statically-interleaved routes (fp8 would be 2x faster on the PE via
DoubleRow but quantizing cd/q/k/v was measured at 2.6-12.6e-2 rel err -- the
logit sigma here is ~3, softmax acts like argmax, errors don't average out):
  A: 3 identity matmuls (w_c * I) accumulate into the scores PSUM (1.28us PE)
  C: DVE pre-combines b3 = r_a*cd_a + cd_piv + r_b*cd_b with two all-f16
     SBUF STT ops (eligible for the DVE 2x/4x fast modes; r = w/w_piv <= 1
     is computed on the host), then ONE pivot-scaled identity matmul adds
     w_piv*b3 to the PSUM (0.43us PE).  The combine depends only on cd, so
     the DVE runs ahead and never blocks the PE.
Softmax normalization is fully on-chip (no DRAM round-trip): the ones-row
of po is reciprocal'd (DVE), broadcast to 64 partitions with a rank-1 PE
matmul (lhsT = ones column), and multiplied into the evicted po (DVE).
Odd heads (outT partitions 64:128) take one SBUF->SBUF partition-shift DMA.

x / xq are pre-transposed on the host so all loads are linear DMAs.
"""

from contextlib import ExitStack

import numpy as np

import concourse.bass as bass
import concourse.mybir as mybir
import concourse.tile as tile
from concourse import bacc
from concourse.masks import make_identity

B, N, D = 4, 2048, 512
HEADS, DH = 8, 64
SCALE = DH ** -0.5
P = 128
IH = N // 2            # query rows handled per core
NCORES = 8
CSHIFT = 5.0           # exp(s - C): keeps unnormalized sums in f16 range
BF = mybir.dt.bfloat16
F16 = mybir.dt.float16
F32 = mybir.dt.float32
MULT = mybir.AluOpType.mult
ADD = mybir.AluOpType.add
EXP = mybir.ActivationFunctionType.Exp

N_WARMUP = 48
# per-head route over the 16 key planes: A = PE 3-pass identity bias,
# C = DVE combine + single pivot identity (see module docstring).
# Measured: STT [128,1024] f16 = 1.28us (no DVE fast mode), PE col rate
# 0.417ns with LDWEIGHTS hidden in back-to-back streams, and heavy DVE
# traffic slows every engine ~20% (SBUF port contention).  B offloads PE
# at half the DVE traffic of C, so the mix is B-heavy with some C.
ROUTE16 = "BBCBBCBBCBBCBBCB"


def build_bass(chan):
    """chan[h] = (a, b, piv): channel order for the pivot-normalized combine."""
    nc = bacc.Bacc(None)
    xt_d = nc.declare_dram_parameter("xt", [D, N], F16, isOutput=False)
    cd_d = nc.declare_dram_parameter("cd", [3, N, IH], F16, isOutput=False)  # [c, j, i]
    wqkv_d = nc.declare_dram_parameter("wqkv", [D, 3 * D], F16, isOutput=False)
    wout_d = nc.declare_dram_parameter("wout", [D, D], F16, isOutput=False)
    bout_d = nc.declare_dram_parameter("bout", [D], F32, isOutput=False)
    relw_d = nc.declare_dram_parameter("relw", [HEADS, 3], F32, isOutput=False)
    relb_d = nc.declare_dram_parameter("relb", [HEADS], F32, isOutput=False)  # pre -C
    rq_d = nc.declare_dram_parameter("rq", [HEADS, 2], F32, isOutput=False)
    out_d = nc.declare_dram_parameter("out", [IH, D], F32, isOutput=True)

    def bcast(ap, parts=P):
        return bass.AP(tensor=ap.tensor, offset=ap.offset, ap=[[0, parts], *ap.ap])

    with ExitStack() as ctx:
        tc = ctx.enter_context(tile.TileContext(nc))
        singles = ctx.enter_context(tc.tile_pool(name="singles", bufs=1))
        cdtp = ctx.enter_context(tc.tile_pool(name="cdtp", bufs=1))
        idp = ctx.enter_context(tc.tile_pool(name="idp", bufs=HEADS))

        # ---- qkv-phase PSUM pool (released before the attention ring) ----
        psq = tc.alloc_tile_pool(name="psq", bufs=4, space="PSUM")

        # identity + a short PE spin keeps the HAM clock hot through the
        # DMA lead-in (pstate ramps to full rate after ~3us of activity)
        ident = singles.tile([P, P], F16)
        make_identity(nc, ident)
        for _ in range(N_WARMUP):
            wu = psq.tile([P, 512], F32, tag="psq")
            nc.tensor.matmul(wu[:, 0:P], lhsT=ident, rhs=ident,
                             start=True, stop=True, skip_group_check=True)

        # ---- constants (SWDGE ring): wqkv's q columns first (they gate
        # the first projection matmul), tiny broadcasts after ----
        wqp = tc.alloc_tile_pool(name="wqp", bufs=1)
        wqkv_sb = wqp.tile([P, 4, 3 * D], F16, tag="wqkv_sb")
        nc.gpsimd.dma_start(
            out=wqkv_sb[:, :, 0:D],
            in_=wqkv_d[:, 0:D].rearrange("(dc p) f -> p dc f", p=P))
        relw_sb = singles.tile([P, HEADS, 3], F32)
        nc.gpsimd.dma_start(out=relw_sb, in_=bcast(relw_d[:, :]))
        relb_sb = singles.tile([P, HEADS], F32)
        nc.gpsimd.dma_start(out=relb_sb, in_=bcast(relb_d[:]))
        rq_sb = singles.tile([P, HEADS, 2], F32)
        nc.gpsimd.dma_start(out=rq_sb, in_=bcast(rq_d[:, :]))
        for fs in range(1, 3):
            nc.gpsimd.dma_start(
                out=wqkv_sb[:, :, fs * D:(fs + 1) * D],
                in_=wqkv_d[:, fs * D:(fs + 1) * D].rearrange(
                    "(dc p) f -> p dc f", p=P))
        # w_c-scaled identities per head, built NOW while ACT is idle --
        # emitted any later they block the qkv evictions in the in-order
        # ACT queue and stall the start of the attention loop by ~10us
        idhs = {}
        for h in range(HEADS):
            idh = idp.tile([P, 3, P], F16, tag="idh")
            for c in range(3):
                nc.scalar.mul(idh[:, c, :], ident, relw_sb[:, h, c:c + 1])
            idhs[h] = idh

        # ---- x / xq: host pre-transposed, plain linear loads (sync ring),
        # quartered so the first projection matmuls start early ----
        qT = singles.tile([P, 4, IH], F16)        # [f%128, fo, i]  (scaled)
        kT = singles.tile([P, 4, N], F16)         # [f%128, fo, j]
        v_sb = singles.tile([P, 16, HEADS, DH + 1], BF)  # [j%128, jt, h, dh | 1]
        nc.vector.memset(v_sb[:, :, :, DH:DH + 1], 1.0)
        xtpool = tc.alloc_tile_pool(name="xtpool", bufs=1)
        xT = xtpool.tile([P, 4, N], F16, tag="xT")      # [d%128, dc, t]
        cdts = []
        for c in range(3):
            cdt = cdtp.tile([P, 16, IH], F16, tag=f"cdt{c}")
            cdts.append(cdt)

        def cd_load(eng, c, ck):
            eng.dma_start(
                out=cdts[c][:, ck * 2:(ck + 1) * 2, :],
                in_=cd_d[c, ck * 256:(ck + 1) * 256, :].rearrange(
                    "(jt p) i -> p jt i", p=P))

        # host orders tokens query-half-first, so xq == xT[:, :, 0:IH] and
        # no separate (duplicate) xq load is needed.  centroid-delta (host
        # pre-transposed, key axis reordered to match xt) goes out in
        # 2-plane chunks spread over three DMA rings (sync / SWDGE /
        # ACT-issued), interleaved so the first chunks of all three
        # channels land just before the attention loop needs them.
        for t4 in range(4):
            nc.sync.dma_start(
                out=xT[:, :, t4 * 512:(t4 + 1) * 512],
                in_=xt_d[:, t4 * 512:(t4 + 1) * 512].rearrange(
                    "(dc p) t -> p dc t", p=P))
        for ck in range(8):
            cd_load(nc.scalar, 0, ck)
            cd_load(nc.scalar, 2, ck)
            cd_load(nc.gpsimd, 1, ck)
        wout_sb = singles.tile([P, 4, D], F16)
        nc.gpsimd.dma_start(out=wout_sb, in_=wout_d.rearrange("(dc p) f -> p dc f", p=P))
        bout_sb = singles.tile([P, D], F32)
        nc.gpsimd.dma_start(out=bout_sb, in_=bcast(bout_d[:]))

        # ---- qkv projection (fp16, f32 PSUM) ----
        def emit_q(t2):
            for fo in range(4):
                ps = psq.tile([P, 512], F32, tag="psq")
                for dc in range(4):
                    nc.tensor.matmul(ps[:, :],
                                     lhsT=wqkv_sb[:, dc, fo * P:(fo + 1) * P],
                                     rhs=xT[:, dc, t2 * 512:(t2 + 1) * 512],
                                     start=(dc == 0), stop=(dc == 3))
                nc.any.tensor_copy(out=qT[:, fo, t2 * 512:(t2 + 1) * 512], in_=ps[:, :])

        def emit_k(t4):
            for fo in range(4):
                ps = psq.tile([P, 512], F32, tag="psq")
                for dc in range(4):
                    nc.tensor.matmul(ps[:, :],
                                     lhsT=wqkv_sb[:, dc, D + fo * P:D + (fo + 1) * P],
                                     rhs=xT[:, dc, t4 * 512:(t4 + 1) * 512],
                                     start=(dc == 0), stop=(dc == 3))
                nc.any.tensor_copy(out=kT[:, fo, t4 * 512:(t4 + 1) * 512], in_=ps[:, :])

        def emit_v(tt):
            ps = psq.tile([P, 512], F32, tag="psq")
            for dc in range(4):
                nc.tensor.matmul(ps[:, :],
                                 lhsT=xT[:, dc, tt * P:(tt + 1) * P],
                                 rhs=wqkv_sb[:, dc, 2 * D:3 * D],
                                 start=(dc == 0), stop=(dc == 3))
            nc.any.tensor_copy(out=v_sb[:, tt, :, 0:DH],
                               in_=ps[:, :].rearrange("p (h d) -> p h d", h=HEADS))

        emit_q(0)
        emit_k(0)
        for tt in range(4):
            emit_v(tt)
        emit_q(1)
        emit_k(1)
        for tt in range(4, 8):
            emit_v(tt)
        emit_k(2)
        for tt in range(8, 12):
            emit_v(tt)
        emit_k(3)
        for tt in range(12, 16):
            emit_v(tt)
        xtpool.release()
        wqp.release()
        psq.release()

        # ---- attention pools ----
        b3p = ctx.enter_context(tc.tile_pool(name="b3p", bufs=3))
        ttp = ctx.enter_context(tc.tile_pool(name="ttp", bufs=2))
        ptp = ctx.enter_context(tc.tile_pool(name="ptp", bufs=3))
        osp = ctx.enter_context(tc.tile_pool(name="osp", bufs=2))
        rrp = ctx.enter_context(tc.tile_pool(name="rrp", bufs=1))
        nrm = ctx.enter_context(tc.tile_pool(name="nrm", bufs=2))
        tmpp = ctx.enter_context(tc.tile_pool(name="tmpp", bufs=2))
        outp = ctx.enter_context(tc.tile_pool(name="outp", bufs=4))
        drp = ctx.enter_context(tc.tile_pool(name="drp", bufs=4, space="DRAM"))
        outT = singles.tile([P, 4, IH], F16)      # [f%128, fo, i]
        ones_sb = singles.tile([P, DH], BF)
        nc.vector.memset(ones_sb, 1.0)
        # PSUM pools allocated last so mid-stream LIFO releases work:
        # release pop (4 banks) -> alloc psop for the output projection.
        # NOTE: score planes MUST be pool-rotated tiles (bufs=2), not manual
        # slots of one big tile -- sub-slice writes of a single PSUM tile get
        # whole-tile dependencies and the pipeline collapses to depth 1.
        ringp = tc.alloc_tile_pool(name="ringp", bufs=2, space="PSUM")
        pop = tc.alloc_tile_pool(name="pop", bufs=2, space="PSUM")

        def emit_normalizer(h, po, chunked=False):
            """po row 64 holds the softmax denominators for head h; divide
            rows 0:64 by it and store into outT (partition-shift DMA for
            odd heads).  The reciprocal runs on a [128,8] layout (a [1,1024]
            DVE reciprocal costs free-size cycles x ~6 uops = 6.5us; the
            transposed shape costs ~0.05us) via two small DMA bounces; the
            whole chain is deferred into the next head so it overlaps."""
            fo, hp = h // 2, (h % 2) * 64
            o_sb = osp.tile([P, IH], BF, tag="osb")
            nc.scalar.copy(out=o_sb[0:DH + 1, :], in_=po[0:DH + 1, :])
            # scatter the sum row across 128 partitions (via a DRAM bounce,
            # SBUF->SBUF partition restructuring fails BIR verification) so
            # the reciprocal costs free-size 8 instead of 1024 (a [1,1024]
            # DVE reciprocal is ~6.5us), then gather back for the broadcast
            dr = drp.tile([IH], BF, tag="dr")
            nc.sync.dma_start(out=dr, in_=o_sb[64:65, :])
            sT = nrm.tile([P, 8], BF, tag="sT")
            nc.sync.dma_start(out=sT, in_=dr.rearrange("(p c) -> p c", p=P))
            rs = nrm.tile([P, 8], BF, tag="rs")
            with nc.allow_low_precision("softmax reciprocal in bf16"):
                nc.vector.reciprocal(out=rs, in_=sT)
            dr2 = drp.tile([IH], BF, tag="dr2")
            nc.sync.dma_start(out=dr2.rearrange("(p c) -> p c", p=P), in_=rs)
            rr = rrp.tile([P, IH], BF, tag="rr")
            nc.sync.dma_start(out=rr[64:65, :], in_=dr2)
            # rank-1 broadcast of the reciprocal row onto 64 partitions,
            # written back over po's (already-evicted) value rows: the next
            # head's po uses the other pool slot, so nothing waits on this
            for half in range(2):
                sl5 = slice(half * 512, (half + 1) * 512)
                nc.tensor.matmul(po[0:64, sl5], lhsT=ones_sb[64:65, :],
                                 rhs=rr[64:65, sl5], start=True, stop=True,
                                 skip_group_check=True)
            if hp == 0:
                if chunked:
                    for tt in range(IH // P):
                        cs = slice(tt * P, (tt + 1) * P)
                        nc.vector.tensor_mul(outT[0:64, fo, cs],
                                             o_sb[0:64, cs], po[0:64, cs])
                else:
                    nc.vector.tensor_mul(outT[0:64, fo, :], o_sb[0:64, :], po[0:64, :])
            else:
                tm = tmpp.tile([P, IH], F16, tag="tmp")
                nc.vector.tensor_mul(tm[0:64, :], o_sb[0:64, :], po[0:64, :])
                nc.sync.dma_start(out=outT[64:128, fo, :], in_=tm[0:64, :])

        pend_norm = None          # (h, po) awaiting normalizer emission
        # last head is even-parity (hp=0): its outT write needs no shift DMA,
        # shortening the tail before the output projection
        for h in [0, 1, 2, 3, 4, 5, 7, 6]:
            fo, hp = h // 2, (h % 2) * 64
            idh = idhs[h]
            ca, cb, cpiv = chan[h]
            po = pop.tile([P, IH], F32, tag="po")
            pT_prev = None
            bpair = None          # (start_jt, [P,2,IH] tile) for batched B pairs
            for jt in range(16):
                route = ROUTE16[jt]
                b3 = None
                if route == 'C':
                    t1 = ttp.tile([P, IH], F16, tag="t1")
                    b3 = b3p.tile([P, IH], F16, tag="b3")
                    nc.vector.scalar_tensor_tensor(
                        out=t1, in0=cdts[ca][:, jt, :], scalar=rq_sb[:, h, 0:1],
                        in1=cdts[cpiv][:, jt, :], op0=MULT, op1=ADD)
                    nc.vector.scalar_tensor_tensor(
                        out=b3, in0=cdts[cb][:, jt, :], scalar=rq_sb[:, h, 1:2],
                        in1=t1, op0=MULT, op1=ADD)
                elif route == 'B':
                    if bpair is not None and bpair[0] + 1 == jt:
                        b3 = bpair[1][:, 1, :]
                        bpair = None
                    elif jt + 1 < 16 and ROUTE16[jt + 1] == 'B':
                        bp = b3p.tile([P, 2, IH], F16, tag="b3")
                        nc.vector.scalar_tensor_tensor(
                            out=bp, in0=cdts[ca][:, jt:jt + 2, :],
                            scalar=rq_sb[:, h, 0:1],
                            in1=cdts[cpiv][:, jt:jt + 2, :], op0=MULT, op1=ADD)
                        bpair = (jt, bp)
                        b3 = bp[:, 0, :]
                    else:
                        b3 = b3p.tile([P, IH], F16, tag="b3s")
                        nc.vector.scalar_tensor_tensor(
                            out=b3, in0=cdts[ca][:, jt, :], scalar=rq_sb[:, h, 0:1],
                            in1=cdts[cpiv][:, jt, :], op0=MULT, op1=ADD)
                ps = ringp.tile([P, IH], F32, tag="ps")
                for half in range(2):
                    sl5 = slice(half * 512, (half + 1) * 512)
                    nc.tensor.matmul(ps[:, sl5],
                                     lhsT=kT[hp:hp + 64, fo, jt * P:(jt + 1) * P],
                                     rhs=qT[hp:hp + 64, fo, sl5],
                                     start=True, stop=False, skip_group_check=True)
                if route == 'C':
                    for half in range(2):
                        sl5 = slice(half * 512, (half + 1) * 512)
                        nc.tensor.matmul(ps[:, sl5], lhsT=idh[:, cpiv, :],
                                         rhs=b3[:, sl5], start=False,
                                         stop=(half == 1), skip_group_check=True)
                elif route == 'B':
                    # w_piv*(r_a cd_a + cd_piv) then w_b*cd_b
                    for half in range(2):
                        sl5 = slice(half * 512, (half + 1) * 512)
                        nc.tensor.matmul(ps[:, sl5], lhsT=idh[:, cpiv, :],
                                         rhs=b3[:, sl5], start=False,
                                         stop=False, skip_group_check=True)
                    for half in range(2):
                        sl5 = slice(half * 512, (half + 1) * 512)
                        nc.tensor.matmul(ps[:, sl5], lhsT=idh[:, cb, :],
                                         rhs=cdts[cb][:, jt, sl5], start=False,
                                         stop=(half == 1), skip_group_check=True)
                else:
                    for c in range(3):
                        for half in range(2):
                            sl5 = slice(half * 512, (half + 1) * 512)
                            nc.tensor.matmul(ps[:, sl5], lhsT=idh[:, c, :],
                                             rhs=cdts[c][:, jt, sl5],
                                             start=False, stop=(c == 2 and half == 1),
                                             skip_group_check=True)
                if jt == 2 and pend_norm is not None:
                    # deferred so the previous head's DMA-bounce reciprocal
                    # chain overlaps this head's first planes
                    emit_normalizer(*pend_norm)
                    pend_norm = None
                pT = ptp.tile([P, IH], BF, tag="pT")
                nc.scalar.activation(out=pT, in_=ps, func=EXP,
                                     bias=relb_sb[:, h:h + 1], scale=1.0)
                if pT_prev is not None:
                    jp, pTp = pT_prev
                    for half in range(2):
                        sl5 = slice(half * 512, (half + 1) * 512)
                        nc.tensor.matmul(po[0:DH + 1, sl5], lhsT=v_sb[:, jp, h, :],
                                         rhs=pTp[:, sl5], start=(jp == 0),
                                         stop=False, skip_group_check=True)
                pT_prev = (jt, pT)
            jp, pTp = pT_prev
            for half in range(2):
                sl5 = slice(half * 512, (half + 1) * 512)
                nc.tensor.matmul(po[0:DH + 1, sl5], lhsT=v_sb[:, jp, h, :],
                                 rhs=pTp[:, sl5], start=False,
                                 stop=(half == 1), skip_group_check=True)
            pend_norm = (h, po)
        emit_normalizer(*pend_norm, chunked=True)
        pop.release()

        # ---- output projection (psop reuses po's freed banks) ----
        psop = tc.alloc_tile_pool(name="psop", bufs=4, space="PSUM")
        for tt in range(IH // P):
            ps = psop.tile([P, D], F32, tag="pso")
            for fo in range(4):
                nc.tensor.matmul(ps[:, :], lhsT=outT[:, fo, tt * P:(tt + 1) * P],
                                 rhs=wout_sb[:, fo, :], start=(fo == 0), stop=(fo == 3))
            osb = outp.tile([P, D], F32, tag="osb")
            nc.vector.scalar_tensor_tensor(out=osb, in0=ps[:, :], scalar=1.0,
                                           in1=bout_sb, op0=MULT, op1=ADD)
            nc.gpsimd.dma_start(out=out_d[tt * P:(tt + 1) * P, :], in_=osb)
        psop.release()
        ringp.release()

    nc.finalize()
    return nc


_CACHE = {}


def _run(in_maps, chan, trace=False, **kw):
    from concourse.bass_utils import run_bass_kernel_spmd
    key = tuple(chan)
    nc = _CACHE.get(key)
    if nc is None:
        nc = build_bass(chan)
        _CACHE[key] = nc
    return run_bass_kernel_spmd(nc, in_maps, list(range(NCORES)), trace=trace, **kw)


def make_in_maps(x, centroid_delta, Wqkv, Wout, bout, rel_w, rel_b):
    f32 = lambda a: np.ascontiguousarray(np.asarray(a, dtype=np.float32))
    f16 = lambda a: np.ascontiguousarray(np.asarray(a, dtype=np.float32).astype(np.float16))
    Wqkv = np.asarray(Wqkv, dtype=np.float32).copy()
    Wqkv[:, :D] *= SCALE          # fold the attention scale into the q columns
    rel_w = np.asarray(rel_w, dtype=np.float32)
    rel_b = np.asarray(rel_b, dtype=np.float32)
    # pivot-normalized channel combine: piv = argmax |w|, r = w_other / w_piv
    chan = []
    rq = np.zeros((HEADS, 2), np.float32)
    for h in range(HEADS):
        piv = int(np.argmax(np.abs(rel_w[h])))
        oth = [c for c in range(3) if c != piv]
        chan.append((oth[0], oth[1], piv))
        rq[h, 0] = rel_w[h, oth[0]] / rel_w[h, piv]
        rq[h, 1] = rel_w[h, oth[1]] / rel_w[h, piv]
    x = f16(x)
    centroid_delta = f16(centroid_delta)
    Wqkv = f16(Wqkv)
    Wout = f16(Wout)
    bout = f32(bout)
    relb_s = f32(rel_b - CSHIFT)
    rel_w = f32(rel_w)
    in_maps = []
    for cid in range(NCORES):
        b, ihf = cid // 2, cid % 2
        sl = slice(ihf * IH, (ihf + 1) * IH)
        oth = slice((1 - ihf) * IH, (2 - ihf) * IH)
        # token (key) order: query-half first -- xq == xt[:, 0:IH] on device,
        # and cd's key axis is permuted identically (softmax is order-
        # invariant over keys, v rows move with their keys)
        xr = np.concatenate([x[b, sl], x[b, oth]], axis=0)
        cdr = np.concatenate([centroid_delta[b, :, sl, sl],
                              centroid_delta[b, :, sl, oth]], axis=2)
        in_maps.append({
            "xt": np.ascontiguousarray(xr.T),
            "cd": np.ascontiguousarray(cdr.transpose(0, 2, 1)),
            "wqkv": Wqkv,
            "wout": Wout,
            "bout": bout,
            "relw": rel_w,
            "relb": relb_s,
            "rq": rq,
        })
    return in_maps, chan


def assemble(results):
    out = np.empty((B, N, D), dtype=np.float32)
    for cid in range(NCORES):
        b, ihf = cid // 2, cid % 2
        out[b, ihf * IH:(ihf + 1) * IH, :] = results[cid]["out"]
    return out


def kernel(x, centroid_delta, Wqkv, Wout, bout, rel_w, rel_b):
    in_maps, chan = make_in_maps(x, centroid_delta, Wqkv, Wout, bout, rel_w, rel_b)
    res = _run(in_maps, chan, trace=False)
    return assemble(res.results)


# revision 30
# speedup vs baseline: 1.1835x; 1.1835x over previous
"""Trainium2 Bass kernel for nn_Attention_37598143709539.

Dense transformer attention with a 1x1-conv relative positional bias:
  qkv = x @ Wqkv ; per-head scores = q k^T * scale + conv1x1(centroid_delta)
  out = softmax(scores) @ v ; final = concat-heads @ Wout + bout

Distribution: pure data-parallel over (batch, query-half) -> 8 cores; core
cid handles batch cid//2, query rows [cid%2*1024, +1024).  Keys/values and
the softmax run over the full 2048-key axis locally, so no collectives are
needed; the host concatenates the 8 output shards.

On-core layout: feature-major (transposed) activations throughout:
  scoresT[j, i] = k_h^T q_h   (key token j on partitions, query i free)
  p = exp(scoresT + biasT - C) (C=5 shift keeps the unnormalized sums in
                                f16 range; it cancels in the normalizer)
  poT[dh|1, i] accumulated with lhsT = [v_h | ones]: the ones column gives
  the softmax normalizer row for free; outT chains into Wout.

Per [128,1024] scores plane the 3-channel conv bias is applied by one of
two statically-interleaved routes (fp8 would be 2x faster on the PE via
DoubleRow but quantizing cd/q/k/v was measured at 2.6-12.6e-2 rel err -- the
logit sigma here is ~3, softmax acts like argmax, errors don't average out):
  A: 3 identity matmuls (w_c * I) accumulate into the scores PSUM (1.28us PE)
  C: DVE pre-combines b3 = r_a*cd_a + cd_piv + r_b*cd_b with two all-f16
     SBUF STT ops (eligible for the DVE 2x/4x fast modes; r = w/w_piv <= 1
     is computed on the host), then ONE pivot-scaled identity matmul adds
     w_piv*b3 to the PSUM (0.43us PE).  The combine depends only on cd, so
     the DVE runs ahead and never blocks the PE.
Softmax normalization is fully on-chip (no DRAM round-trip): the ones-row
of po is reciprocal'd (DVE), broadcast to 64 partitions with a rank-1 PE
matmul (lhsT = ones column), and multiplied into the evicted po (DVE).
Odd heads (outT partitions 64:128) take one SBUF->SBUF partition-shift DMA.

x / xq are pre-transposed on the host so all loads are linear DMAs.
"""

from contextlib import ExitStack

import numpy as np

import concourse.bass as bass
import concourse.mybir as mybir
import concourse.tile as tile
from concourse import bacc
from concourse.masks import make_identity

B, N, D = 4, 2048, 512
HEADS, DH = 8, 64
SCALE = DH ** -0.5
P = 128
IH = N // 2            # query rows handled per core
NCORES = 8
CSHIFT = 5.0           # exp(s - C): keeps unnormalized sums in f16 range
BF = mybir.dt.bfloat16
F16 = mybir.dt.float16
F32 = mybir.dt.float32
MULT = mybir.AluOpType.mult
ADD = mybir.AluOpType.add
EXP = mybir.ActivationFunctionType.Exp

N_WARMUP = 48
# per-head route over the 16 key planes: A = PE 3-pass identity bias,
# C = DVE combine + single pivot identity (see module docstring).
# Measured: STT [128,1024] f16 = 1.28us (no DVE fast mode), PE col rate
# 0.417ns with LDWEIGHTS hidden in back-to-back streams, and heavy DVE
# traffic slows every engine ~20% (SBUF port contention).  B offloads PE
# at half the DVE traffic of C, so the mix is B-heavy with some C.
ROUTE16 = "BBCBBCBBCBBCBBCB"


def build_bass(chan):
    """chan[h] = (a, b, piv): channel order for the pivot-normalized combine."""
    nc = bacc.Bacc(None)
    xt_d = nc.declare_dram_parameter("xt", [D, N], F16, isOutput=False)
    cd_d = nc.declare_dram_parameter("cd", [3, N, IH], F16, isOutput=False)  # [c, j, i]
    wqkv_d = nc.declare_dram_parameter("wqkv", [D, 3 * D], F16, isOutput=False)
    wout_d = nc.declare_dram_parameter("wout", [D, D], F16, isOutput=False)
    bout_d = nc.declare_dram_parameter("bout", [D], F32, isOutput=False)
    relw_d = nc.declare_dram_parameter("relw", [HEADS, 3], F32, isOutput=False)
    relb_d = nc.declare_dram_parameter("relb", [HEADS], F32, isOutput=False)  # pre -C
    rq_d = nc.declare_dram_parameter("rq", [HEADS, 2], F32, isOutput=False)
    out_d = nc.declare_dram_parameter("out", [IH, D], F32, isOutput=True)

    def bcast(ap, parts=P):
        return bass.AP(tensor=ap.tensor, offset=ap.offset, ap=[[0, parts], *ap.ap])

    with ExitStack() as ctx:
        tc = ctx.enter_context(tile.TileContext(nc))
        singles = ctx.enter_context(tc.tile_pool(name="singles", bufs=1))
        cdtp = ctx.enter_context(tc.tile_pool(name="cdtp", bufs=1))
        idp = ctx.enter_context(tc.tile_pool(name="idp", bufs=HEADS))

        # ---- qkv-phase PSUM pool (released before the attention ring) ----
        psq = tc.alloc_tile_pool(name="psq", bufs=4, space="PSUM")

        # identity + a short PE spin keeps the HAM clock hot through the
        # DMA lead-in (pstate ramps to full rate after ~3us of activity)
        ident = singles.tile([P, P], F16)
        make_identity(nc, ident)
        for _ in range(N_WARMUP):
            wu = psq.tile([P, 512], F32, tag="psq")
            nc.tensor.matmul(wu[:, 0:P], lhsT=ident, rhs=ident,
                             start=True, stop=True, skip_group_check=True)

        # ---- constants (SWDGE ring): relw first (it gates the idh
        # identity builds on the otherwise-idle ACT), then wqkv's q
        # columns (they gate the first projection matmul) ----
        relw_sb = singles.tile([P, HEADS, 3], F32)
        nc.gpsimd.dma_start(out=relw_sb, in_=bcast(relw_d[:, :]))
        wqp = tc.alloc_tile_pool(name="wqp", bufs=1)
        wqkv_sb = wqp.tile([P, 4, 3 * D], F16, tag="wqkv_sb")
        nc.gpsimd.dma_start(
            out=wqkv_sb[:, :, 0:D],
            in_=wqkv_d[:, 0:D].rearrange("(dc p) f -> p dc f", p=P))
        relb_sb = singles.tile([P, HEADS], F32)
        nc.gpsimd.dma_start(out=relb_sb, in_=bcast(relb_d[:]))
        rq_sb = singles.tile([P, HEADS, 2], F32)
        nc.gpsimd.dma_start(out=rq_sb, in_=bcast(rq_d[:, :]))
        for fs in range(1, 3):
            nc.gpsimd.dma_start(
                out=wqkv_sb[:, :, fs * D:(fs + 1) * D],
                in_=wqkv_d[:, fs * D:(fs + 1) * D].rearrange(
                    "(dc p) f -> p dc f", p=P))
        # w_c-scaled identities per head, built NOW while ACT is idle --
        # emitted any later they block the qkv evictions in the in-order
        # ACT queue and stall the start of the attention loop by ~10us
        idhs = {}
        for h in range(HEADS):
            idh = idp.tile([P, 3, P], F16, tag="idh")
            for c in range(3):
                nc.scalar.mul(idh[:, c, :], ident, relw_sb[:, h, c:c + 1])
            idhs[h] = idh

        # ---- x / xq: host pre-transposed, plain linear loads (sync ring),
        # quartered so the first projection matmuls start early ----
        qT = singles.tile([P, 4, IH], F16)        # [f%128, fo, i]  (scaled)
        kT = singles.tile([P, 4, N], F16)         # [f%128, fo, j]
        v_sb = singles.tile([P, 16, HEADS, DH + 1], BF)  # [j%128, jt, h, dh | 1]
        nc.vector.memset(v_sb[:, :, :, DH:DH + 1], 1.0)
        xtpool = tc.alloc_tile_pool(name="xtpool", bufs=1)
        xT = xtpool.tile([P, 4, N], F16, tag="xT")      # [d%128, dc, t]
        cdts = []
        for c in range(3):
            cdt = cdtp.tile([P, 16, IH], F16, tag=f"cdt{c}")
            cdts.append(cdt)

        def cd_load(eng, c, ck):
            eng.dma_start(
                out=cdts[c][:, ck * 2:(ck + 1) * 2, :],
                in_=cd_d[c, ck * 256:(ck + 1) * 256, :].rearrange(
                    "(jt p) i -> p jt i", p=P))

        # host orders tokens query-half-first, so xq == xT[:, :, 0:IH] and
        # no separate (duplicate) xq load is needed.  centroid-delta (host
        # pre-transposed, key axis reordered to match xt) goes out in
        # 2-plane chunks spread over three DMA rings (sync / SWDGE /
        # ACT-issued), interleaved so the first chunks of all three
        # channels land just before the attention loop needs them.
        for t4 in range(4):
            nc.sync.dma_start(
                out=xT[:, :, t4 * 512:(t4 + 1) * 512],
                in_=xt_d[:, t4 * 512:(t4 + 1) * 512].rearrange(
                    "(dc p) t -> p dc t", p=P))
        for ck in range(8):
            cd_load(nc.scalar, 0, ck)
            cd_load(nc.scalar, 2, ck)
            cd_load(nc.gpsimd, 1, ck)
        wout_sb = singles.tile([P, 4, D], F16)
        nc.gpsimd.dma_start(out=wout_sb, in_=wout_d.rearrange("(dc p) f -> p dc f", p=P))
        bout_sb = singles.tile([P, D], F32)
        nc.gpsimd.dma_start(out=bout_sb, in_=bcast(bout_d[:]))

        # ---- qkv projection (fp16, f32 PSUM) ----
        def emit_q(t2):
            for fo in range(4):
                ps = psq.tile([P, 512], F32, tag="psq")
                for dc in range(4):
                    nc.tensor.matmul(ps[:, :],
                                     lhsT=wqkv_sb[:, dc, fo * P:(fo + 1) * P],
                                     rhs=xT[:, dc, t2 * 512:(t2 + 1) * 512],
                                     start=(dc == 0), stop=(dc == 3))
                nc.any.tensor_copy(out=qT[:, fo, t2 * 512:(t2 + 1) * 512], in_=ps[:, :])

        def emit_k(t4):
            for fo in range(4):
                ps = psq.tile([P, 512], F32, tag="psq")
                for dc in range(4):
                    nc.tensor.matmul(ps[:, :],
                                     lhsT=wqkv_sb[:, dc, D + fo * P:D + (fo + 1) * P],
                                     rhs=xT[:, dc, t4 * 512:(t4 + 1) * 512],
                                     start=(dc == 0), stop=(dc == 3))
                nc.any.tensor_copy(out=kT[:, fo, t4 * 512:(t4 + 1) * 512], in_=ps[:, :])

        def emit_v(tt):
            ps = psq.tile([P, 512], F32, tag="psq")
            for dc in range(4):
                nc.tensor.matmul(ps[:, :],
                                 lhsT=xT[:, dc, tt * P:(tt + 1) * P],
                                 rhs=wqkv_sb[:, dc, 2 * D:3 * D],
                                 start=(dc == 0), stop=(dc == 3))
            nc.any.tensor_copy(out=v_sb[:, tt, :, 0:DH],
                               in_=ps[:, :].rearrange("p (h d) -> p h d", h=HEADS))

        emit_q(0)
        emit_k(0)
        for tt in range(4):
            emit_v(tt)
        emit_q(1)
        emit_k(1)
        for tt in range(4, 8):
            emit_v(tt)
        emit_k(2)
        for tt in range(8, 12):
            emit_v(tt)
        emit_k(3)
        for tt in range(12, 16):
            emit_v(tt)
        xtpool.release()
        wqp.release()
        psq.release()

        # ---- attention pools ----
        b3p = ctx.enter_context(tc.tile_pool(name="b3p", bufs=3))
        ttp = ctx.enter_context(tc.tile_pool(name="ttp", bufs=2))
        ptp = ctx.enter_context(tc.tile_pool(name="ptp", bufs=3))
        osp = ctx.enter_context(tc.tile_pool(name="osp", bufs=2))
        rrp = ctx.enter_context(tc.tile_pool(name="rrp", bufs=1))
        nrm = ctx.enter_context(tc.tile_pool(name="nrm", bufs=2))
        tmpp = ctx.enter_context(tc.tile_pool(name="tmpp", bufs=2))
        outp = ctx.enter_context(tc.tile_pool(name="outp", bufs=4))
        drp = ctx.enter_context(tc.tile_pool(name="drp", bufs=4, space="DRAM"))
        outT = singles.tile([P, 4, IH], F16)      # [f%128, fo, i]
        ones_sb = singles.tile([P, DH], BF)
        nc.vector.memset(ones_sb, 1.0)
        # PSUM pools allocated last so mid-stream LIFO releases work:
        # release pop (4 banks) -> alloc psop for the output projection.
        # NOTE: score planes MUST be pool-rotated tiles (bufs=2), not manual
        # slots of one big tile -- sub-slice writes of a single PSUM tile get
        # whole-tile dependencies and the pipeline collapses to depth 1.
        ringp = tc.alloc_tile_pool(name="ringp", bufs=2, space="PSUM")
        pop = tc.alloc_tile_pool(name="pop", bufs=2, space="PSUM")

        def emit_normalizer(h, po, chunked=False):
            """po row 64 holds the softmax denominators for head h; divide
            rows 0:64 by it and store into outT (partition-shift DMA for
            odd heads).  The reciprocal runs on a [128,8] layout (a [1,1024]
            DVE reciprocal costs free-size cycles x ~6 uops = 6.5us; the
            transposed shape costs ~0.05us) via two small DMA bounces; the
            whole chain is deferred into the next head so it overlaps."""
            fo, hp = h // 2, (h % 2) * 64
            o_sb = osp.tile([P, IH], BF, tag="osb")
            nc.scalar.copy(out=o_sb[0:DH + 1, :], in_=po[0:DH + 1, :])
            # scatter the sum row across 128 partitions (via a DRAM bounce,
            # SBUF->SBUF partition restructuring fails BIR verification) so
            # the reciprocal costs free-size 8 instead of 1024 (a [1,1024]
            # DVE reciprocal is ~6.5us), then gather back for the broadcast
            dr = drp.tile([IH], BF, tag="dr")
            nc.sync.dma_start(out=dr, in_=o_sb[64:65, :])
            sT = nrm.tile([P, 8], BF, tag="sT")
            nc.sync.dma_start(out=sT, in_=dr.rearrange("(p c) -> p c", p=P))
            rs = nrm.tile([P, 8], BF, tag="rs")
            with nc.allow_low_precision("softmax reciprocal in bf16"):
                nc.vector.reciprocal(out=rs, in_=sT)
            dr2 = drp.tile([IH], BF, tag="dr2")
            nc.sync.dma_start(out=dr2.rearrange("(p c) -> p c", p=P), in_=rs)
            rr = rrp.tile([P, IH], BF, tag="rr")
            nc.sync.dma_start(out=rr[64:65, :], in_=dr2)
            # rank-1 broadcast of the reciprocal row onto 64 partitions,
            # written back over po's (already-evicted) value rows: the next
            # head's po uses the other pool slot, so nothing waits on this
            for half in range(2):
                sl5 = slice(half * 512, (half + 1) * 512)
                nc.tensor.matmul(po[0:64, sl5], lhsT=ones_sb[64:65, :],
                                 rhs=rr[64:65, sl5], start=True, stop=True,
                                 skip_group_check=True)
            if hp == 0:
                if chunked:
                    for tt in range(IH // P):
                        cs = slice(tt * P, (tt + 1) * P)
                        nc.vector.tensor_mul(outT[0:64, fo, cs],
                                             o_sb[0:64, cs], po[0:64, cs])
                else:
                    nc.vector.tensor_mul(outT[0:64, fo, :], o_sb[0:64, :], po[0:64, :])
            else:
                tm = tmpp.tile([P, IH], F16, tag="tmp")
                nc.vector.tensor_mul(tm[0:64, :], o_sb[0:64, :], po[0:64, :])
                nc.sync.dma_start(out=outT[64:128, fo, :], in_=tm[0:64, :])

        pend_norm = None          # (h, po) awaiting normalizer emission
        # last head is even-parity (hp=0): its outT write needs no shift DMA,
        # shortening the tail before the output projection
        for h in [0, 1, 2, 3, 4, 5, 7, 6]:
            fo, hp = h // 2, (h % 2) * 64
            idh = idhs[h]
            ca, cb, cpiv = chan[h]
            po = pop.tile([P, IH], F32, tag="po")
            pT_prev = None
            bpair = None          # (start_jt, [P,2,IH] tile) for batched B pairs
            for jt in range(16):
                route = ROUTE16[jt]
                b3 = None
                if route == 'C':
                    t1 = ttp.tile([P, IH], F16, tag="t1")
                    b3 = b3p.tile([P, IH], F16, tag="b3")
                    nc.vector.scalar_tensor_tensor(
                        out=t1, in0=cdts[ca][:, jt, :], scalar=rq_sb[:, h, 0:1],
                        in1=cdts[cpiv][:, jt, :], op0=MULT, op1=ADD)
                    nc.vector.scalar_tensor_tensor(
                        out=b3, in0=cdts[cb][:, jt, :], scalar=rq_sb[:, h, 1:2],
                        in1=t1, op0=MULT, op1=ADD)
                elif route == 'B':
                    if bpair is not None and bpair[0] + 1 == jt:
                        b3 = bpair[1][:, 1, :]
                        bpair = None
                    elif jt + 1 < 16 and ROUTE16[jt + 1] == 'B':
                        bp = b3p.tile([P, 2, IH], F16, tag="b3")
                        nc.vector.scalar_tensor_tensor(
                            out=bp, in0=cdts[ca][:, jt:jt + 2, :],
                            scalar=rq_sb[:, h, 0:1],
                            in1=cdts[cpiv][:, jt:jt + 2, :], op0=MULT, op1=ADD)
                        bpair = (jt, bp)
                        b3 = bp[:, 0, :]
                    else:
                        b3 = b3p.tile([P, IH], F16, tag="b3s")
                        nc.vector.scalar_tensor_tensor(
                            out=b3, in0=cdts[ca][:, jt, :], scalar=rq_sb[:, h, 0:1],
                            in1=cdts[cpiv][:, jt, :], op0=MULT, op1=ADD)
                ps = ringp.tile([P, IH], F32, tag="ps")
                for half in range(2):
                    sl5 = slice(half * 512, (half + 1) * 512)
                    nc.tensor.matmul(ps[:, sl5],
                                     lhsT=kT[hp:hp + 64, fo, jt * P:(jt + 1) * P],
                                     rhs=qT[hp:hp + 64, fo, sl5],
                                     start=True, stop=False, skip_group_check=True)
                if route == 'C':
                    for half in range(2):
                        sl5 = slice(half * 512, (half + 1) * 512)
                        nc.tensor.matmul(ps[:, sl5], lhsT=idh[:, cpiv, :],
                                         rhs=b3[:, sl5], start=False,
                                         stop=(half == 1), skip_group_check=True)
                elif route == 'B':
                    # w_piv*(r_a cd_a + cd_piv) then w_b*cd_b
                    for half in range(2):
                        sl5 = slice(half * 512, (half + 1) * 512)
                        nc.tensor.matmul(ps[:, sl5], lhsT=idh[:, cpiv, :],
                                         rhs=b3[:, sl5], start=False,
                                         stop=False, skip_group_check=True)
                    for half in range(2):
                        sl5 = slice(half * 512, (half + 1) * 512)
                        nc.tensor.matmul(ps[:, sl5], lhsT=idh[:, cb, :],
                                         rhs=cdts[cb][:, jt, sl5], start=False,
                                         stop=(half == 1), skip_group_check=True)
                else:
                    for c in range(3):
                        for half in range(2):
                            sl5 = slice(half * 512, (half + 1) * 512)
                            nc.tensor.matmul(ps[:, sl5], lhsT=idh[:, c, :],
                                             rhs=cdts[c][:, jt, sl5],
                                             start=False, stop=(c == 2 and half == 1),
                                             skip_group_check=True)
                if jt == 2 and pend_norm is not None:
                    # deferred so the previous head's DMA-bounce reciprocal
                    # chain overlaps this head's first planes
                    emit_normalizer(*pend_norm)
                    pend_norm = None
                pT = ptp.tile([P, IH], BF, tag="pT")
                nc.scalar.activation(out=pT, in_=ps, func=EXP,
                                     bias=relb_sb[:, h:h + 1], scale=1.0)
                if pT_prev is not None:
                    jp, pTp = pT_prev
                    for half in range(2):
                        sl5 = slice(half * 512, (half + 1) * 512)
                        nc.tensor.matmul(po[0:DH + 1, sl5], lhsT=v_sb[:, jp, h, :],
                                         rhs=pTp[:, sl5], start=(jp == 0),
                                         stop=False, skip_group_check=True)
                pT_prev = (jt, pT)
            jp, pTp = pT_prev
            for half in range(2):
                sl5 = slice(half * 512, (half + 1) * 512)
                nc.tensor.matmul(po[0:DH + 1, sl5], lhsT=v_sb[:, jp, h, :],
                                 rhs=pTp[:, sl5], start=False,
                                 stop=(half == 1), skip_group_check=True)
            pend_norm = (h, po)
        emit_normalizer(*pend_norm, chunked=True)
        pop.release()

        # ---- output projection (psop reuses po's freed banks) ----
        psop = tc.alloc_tile_pool(name="psop", bufs=4, space="PSUM")
        for tt in range(IH // P):
            ps = psop.tile([P, D], F32, tag="pso")
            for fo in range(4):
                nc.tensor.matmul(ps[:, :], lhsT=outT[:, fo, tt * P:(tt + 1) * P],
                                 rhs=wout_sb[:, fo, :], start=(fo == 0), stop=(fo == 3))
            osb = outp.tile([P, D], F32, tag="osb")
            nc.vector.scalar_tensor_tensor(out=osb, in0=ps[:, :], scalar=1.0,
                                           in1=bout_sb, op0=MULT, op1=ADD)
            nc.gpsimd.dma_start(out=out_d[tt * P:(tt + 1) * P, :], in_=osb)
        psop.release()
        ringp.release()

    nc.finalize()
    return nc


_CACHE = {}


def _run(in_maps, chan, trace=False, **kw):
    from concourse.bass_utils import run_bass_kernel_spmd
    key = tuple(chan)
    nc = _CACHE.get(key)
    if nc is None:
        nc = build_bass(chan)
        _CACHE[key] = nc
    return run_bass_kernel_spmd(nc, in_maps, list(range(NCORES)), trace=trace, **kw)


def make_in_maps(x, centroid_delta, Wqkv, Wout, bout, rel_w, rel_b):
    f32 = lambda a: np.ascontiguousarray(np.asarray(a, dtype=np.float32))
    f16 = lambda a: np.ascontiguousarray(np.asarray(a, dtype=np.float32).astype(np.float16))
    Wqkv = np.asarray(Wqkv, dtype=np.float32).copy()
    Wqkv[:, :D] *= SCALE          # fold the attention scale into the q columns
    rel_w = np.asarray(rel_w, dtype=np.float32)
    rel_b = np.asarray(rel_b, dtype=np.float32)
    # pivot-normalized channel combine: piv = argmax |w|, r = w_other / w_piv
    chan = []
    rq = np.zeros((HEADS, 2), np.float32)
    for h in range(HEADS):
        piv = int(np.argmax(np.abs(rel_w[h])))
        oth = [c for c in range(3) if c != piv]
        chan.append((oth[0], oth[1], piv))
        rq[h, 0] = rel_w[h, oth[0]] / rel_w[h, piv]
        rq[h, 1] = rel_w[h, oth[1]] / rel_w[h, piv]
    x = f16(x)
    centroid_delta = f16(centroid_delta)
    Wqkv = f16(Wqkv)
    Wout = f16(Wout)
    bout = f32(bout)
    relb_s = f32(rel_b - CSHIFT)
    rel_w = f32(rel_w)
    in_maps = []
    for cid in range(NCORES):
        b, ihf = cid // 2, cid % 2
        sl = slice(ihf * IH, (ihf + 1) * IH)
        oth = slice((1 - ihf) * IH, (2 - ihf) * IH)
        # token (key) order: query-half first -- xq == xt[:, 0:IH] on device,
        # and cd's key axis is permuted identically (softmax is order-
        # invariant over keys, v rows move with their keys)
        xr = np.concatenate([x[b, sl], x[b, oth]], axis=0)
        cdr = np.concatenate([centroid_delta[b, :, sl, sl],
                              centroid_delta[b, :, sl, oth]], axis=2)
        in_maps.append({
            "xt": np.ascontiguousarray(xr.T),
            "cd": np.ascontiguousarray(cdr.transpose(0, 2, 1)),
            "wqkv": Wqkv,
            "wout": Wout,
            "bout": bout,
            "relw": rel_w,
            "relb": relb_s,
            "rq": rq,
        })
    return in_maps, chan


def assemble(results):
    out = np.empty((B, N, D), dtype=np.float32)
    for cid in range(NCORES):
        b, ihf = cid // 2, cid % 2
        out[b, ihf * IH:(ihf + 1) * IH, :] = results[cid]["out"]
    return out


def kernel(x, centroid_delta, Wqkv, Wout, bout, rel_w, rel_b):
    in_maps, chan = make_in_maps(x, centroid_delta, Wqkv, Wout, bout, rel_w, rel_b)
    res = _run(in_maps, chan, trace=False)
    return assemble(res.results)
